# revision 7
# baseline (speedup 1.0000x reference)
"""Hyperbolic (Poincare-ball) average pooling 1D — Trainium2 Bass kernel.

Problem: x (16, 256, 16384) f32, kernel=stride=4, manifold dim = channels (256).
Math (c=1), per window position:
    n2   = sum_C x^2                     (per input position)
    r    = 1/(1-n2)                      (gamma*xK = 2*r*x ; gamma = 2r-1)
    num  = sum_j r_j x_j  (window of 4)  ; den = sum_j r_j ; D = den - 2
    mK   = num/D ; out = mK / (1 + sqrt(1 - |mK|^2/...)) which reduces to
    out  = num * g,  g = 1/(D + sqrt(D^2 - s)),  s = sum_C num^2

Strategy: shard over batch (2 per core, 8 cores). Host pre-transposes each
core's slice to (positions, channels) with a ones-column appended, so on
device every per-position scalar lives on a partition (cheap, batched along
free) and the non-overlapping window-sum is a PE matmul with an r-scaled
band matrix (f32r, 1 cyc/col).  Reciprocals via exp(-ln(.)) on ScalarE
(single act table set; ScalarE Reciprocal is banned for accuracy).
"""

import sys

sys.path.insert(0, "/opt/trn_rl_repo")

import copy
import numpy as np

import concourse.bass as bass
import concourse.mybir as mybir
from concourse import tile
from concourse.bass_utils import run_bass_kernel_spmd
from contextlib import ExitStack

F32 = mybir.dt.float32
F32R = mybir.dt.float32r

B, C, L = 16, 256, 16384
KERN = 4
T = L // KERN            # 4096 out positions per batch row
N_CORES = 8
B_PER = B // N_CORES     # 2
POS = B_PER * L          # 32768 input positions per core
OPOS = POS // KERN       # 8192 out positions per core
CPC = C + 2              # 258: channels + ones column + zero pad (f32r needs even N)
Q = 8                    # q-slots per x-tile
TILE_POS = 128 * Q       # 1024 input positions per x-tile
N_TILES = POS // TILE_POS  # 32
SB = 8                   # x-tiles per sub-batch (g-chain batching)
RG = 4                   # x-tiles per r-group (r-math batching)

AF = mybir.ActivationFunctionType
ALU = mybir.AluOpType


def _split_multi_waits(nc, max_waits=1):
    """walrus in this container rejects >1 sync-wait on one instruction
    (setupSyncWait 'Too many sync wait commands'); split extras into
    preceding single-wait NOPs on the same engine."""
    n_new = 0
    for bb in nc.m.functions[0].blocks:
        new_list = []
        for inst in bb.instructions:
            si = getattr(inst, "sync_info", None)
            if si is not None and si.on_wait and len(si.on_wait) > max_waits:
                extra = si.on_wait[:-max_waits]
                si_keep = si.on_wait[-max_waits:]
                for w in extra:
                    nop = mybir.InstNoOp(
                        name=f"{inst.name}-wsplit{n_new}", ins=[], outs=[]
                    )
                    nop.engine = inst.engine
                    nsi = copy.deepcopy(si)
                    nsi.on_wait = [w]
                    nsi.on_update = []
                    nop.sync_info = nsi
                    new_list.append(nop)
                    n_new += 1
                si.on_wait = si_keep
            new_list.append(inst)
        bb.instructions = new_list
    return n_new


def _register_const_ap(nc, value):
    t = nc.alloc_sbuf_tensor(f"const-float32-{value}", [128, 1], F32)
    nc.gpsimd.memset(t.ap(), value)
    nc.const_aps.aps[(F32, value)] = t.ap()


def build_nc(split_waits=True):
    nc = bass.Bass()
    _register_const_ap(nc, 2.0)
    _register_const_ap(nc, -2.0)
    nc.all_engine_barrier()
    xt = nc.declare_dram_parameter("xt", [POS, CPC], F32, isOutput=False)
    mask = nc.declare_dram_parameter("mask", [128, Q * 128], F32, isOutput=False)
    out = nc.declare_dram_parameter("out", [OPOS, C], F32, isOutput=True)

    with tile.TileContext(nc) as tc:
        with ExitStack() as ctx:
            xpool = ctx.enter_context(tc.tile_pool(name="x", bufs=6))
            sqpool = ctx.enter_context(tc.tile_pool(name="sq", bufs=3))
            wrpool = ctx.enter_context(tc.tile_pool(name="wr", bufs=3))
            numpool = ctx.enter_context(tc.tile_pool(name="num", bufs=SB + 2))
            sqdpool = ctx.enter_context(tc.tile_pool(name="sqd", bufs=3))
            stpool = ctx.enter_context(tc.tile_pool(name="st", bufs=2))
            opool = ctx.enter_context(tc.tile_pool(name="o", bufs=3))
            mkpool = ctx.enter_context(tc.tile_pool(name="mk", bufs=1))
            pspool = ctx.enter_context(tc.tile_pool(name="ps", bufs=6, space="PSUM"))

            mask_t = mkpool.tile([128, Q * 128], F32, tag="mask")
            nc.sync.dma_start(mask_t[:], mask[:, :])

            for sb in range(N_TILES // SB):
                d_s = stpool.tile([128, 2 * SB], F32, tag="d")
                s_s = stpool.tile([128, 2 * SB], F32, tag="s")
                num_tiles = []
                for rg in range(SB // RG):
                    m_g = stpool.tile([128, Q * RG], F32, tag="m")
                    xts = []
                    for j in range(RG):
                        i = sb * SB + rg * RG + j
                        x_t = xpool.tile([128, Q, CPC], F32R, tag="x")
                        nc.sync.dma_start(
                            x_t[:],
                            xt[i * TILE_POS : (i + 1) * TILE_POS, :]
                            .bitcast(F32R)
                            .rearrange("(q p) c -> p q c", p=128),
                        )
                        sq_t = sqpool.tile([128, Q, CPC], F32, tag="sq")
                        nc.scalar.activation(sq_t[:], x_t[:].bitcast(F32), AF.Square)
                        # m = n2 + 1 (ones column squared contributes 1)
                        nc.vector.tensor_reduce(
                            m_g[:, j * Q : (j + 1) * Q],
                            sq_t[:],
                            axis=mybir.AxisListType.X,
                            op=ALU.add,
                        )
                        xts.append(x_t)
                    # r = 1/(1-n2) = exp(-ln(2-m)), batched over RG tiles
                    ln_g = stpool.tile([128, Q * RG], F32, tag="ln")
                    nc.scalar.activation(
                        ln_g[:], m_g[:], AF.Ln, bias=2.0, scale=-1.0
                    )
                    r_g = stpool.tile([128, Q * RG], F32, tag="r")
                    nc.scalar.activation(r_g[:], ln_g[:], AF.Exp, scale=-1.0)

                    for j in range(RG):
                        i = sb * SB + rg * RG + j
                        jj = rg * RG + j  # index within sub-batch
                        x_t = xts[j]
                        # Wr[p, q*32+t] = mask[p,t] * r[p,q]
                        wr_t = wrpool.tile([128, Q, 128], F32R, tag="wr")
                        r_b = (
                            r_g[:, j * Q : (j + 1) * Q]
                            .rearrange("p (q o) -> p q o", o=1)
                            .broadcast_to([128, Q, 128])
                        )
                        nc.vector.tensor_tensor(
                            out=wr_t[:],
                            in0=mask_t[:].rearrange("p (q t) -> p q t", q=Q),
                            in1=r_b,
                            op=ALU.mult,
                        )
                        # 2 accumulation groups of 4 band matmuls -> (128, 258)
                        num_t = numpool.tile([128, 2, 257], F32, tag="num")
                        for bk in range(2):
                            ps = pspool.tile([128, CPC], F32, tag="ps")
                            for ql in range(4):
                                q = 4 * bk + ql
                                nc.tensor.matmul(
                                    ps[:, :],
                                    wr_t[:, q, :],
                                    x_t[:, q, :],
                                    start=(ql == 0),
                                    stop=(ql == 3),
                                )
                            # evacuate PSUM -> SBUF (num||den)
                            nc.scalar.copy(num_t[:, bk, :], ps[:, 0:257])
                            # s = sum_C num^2 (Square + accum_out), from PSUM
                            sqd = sqdpool.tile([128, 256], F32, tag="sqd")
                            nc.scalar.activation(
                                sqd[:],
                                ps[:, 0:256],
                                AF.Square,
                                accum_out=s_s[:, 2 * jj + bk : 2 * jj + bk + 1],
                            )
                        # D = den - 2 (den = matmul of ones column)
                        nc.scalar.activation(
                            d_s[:, 2 * jj : 2 * jj + 2],
                            num_t[:, :, 256],
                            AF.Identity,
                            bias=-2.0,
                        )
                        num_tiles.append(num_t)

                # g = 1/(D + sqrt(D^2 - s)) = exp(-ln(D + exp(0.5*ln(D^2-s))))
                d2 = stpool.tile([128, 2 * SB], F32, tag="d2")
                nc.scalar.activation(d2[:], d_s[:], AF.Square)
                qq = stpool.tile([128, 2 * SB], F32, tag="qq")
                nc.vector.tensor_tensor(out=qq[:], in0=d2[:], in1=s_s[:], op=ALU.subtract)
                lnq = stpool.tile([128, 2 * SB], F32, tag="lnq")
                nc.scalar.activation(lnq[:], qq[:], AF.Ln)
                u = stpool.tile([128, 2 * SB], F32, tag="u")
                nc.scalar.activation(u[:], lnq[:], AF.Exp, scale=0.5)
                du = stpool.tile([128, 2 * SB], F32, tag="du")
                nc.vector.tensor_tensor(out=du[:], in0=d_s[:], in1=u[:], op=ALU.add)
                lnd = stpool.tile([128, 2 * SB], F32, tag="lnd")
                nc.scalar.activation(lnd[:], du[:], AF.Ln)
                g_s = stpool.tile([128, 2 * SB], F32, tag="g")
                nc.scalar.activation(g_s[:], lnd[:], AF.Exp, scale=-1.0)

                for jj in range(SB):
                    i = sb * SB + jj
                    num_t = num_tiles[jj]
                    o_t = opool.tile([128, 2 * 256], F32, tag="o")
                    nc.vector.tensor_scalar_mul(
                        o_t[:, 0:256],
                        num_t[:, 0, 0:256],
                        g_s[:, 2 * jj : 2 * jj + 1],
                    )
                    nc.vector.tensor_scalar_mul(
                        o_t[:, 256:512],
                        num_t[:, 1, 0:256],
                        g_s[:, 2 * jj + 1 : 2 * jj + 2],
                    )
                    nc.sync.dma_start(
                        out[i * 256 : (i + 1) * 256, :].rearrange(
                            "(b p) c -> p b c", p=128
                        ),
                        o_t[:].rearrange("p (b c) -> p b c", b=2),
                    )

    if split_waits:
        _split_multi_waits(nc)
    return nc


_NC_CACHE = None


def _get_nc():
    global _NC_CACHE
    if _NC_CACHE is None:
        _NC_CACHE = build_nc()
    return _NC_CACHE


def _make_mask():
    m = np.zeros((128, Q * 128), dtype=np.float32)
    p = np.arange(128)
    for q in range(Q):
        m[p, q * 128 + 32 * (q % 4) + p // 4] = 1.0
    return m


def prepare_core_inputs(x):
    """x: (16, 256, 16384) f32 -> list of per-core input dicts."""
    mask = _make_mask()
    in_maps = []
    for k in range(N_CORES):
        xs = x[k * B_PER : (k + 1) * B_PER]  # (2, 256, L)
        xt = np.empty((POS, CPC), dtype=np.float32)
        xt[:, :C] = xs.transpose(0, 2, 1).reshape(POS, C)
        xt[:, C] = 1.0
        xt[:, C + 1] = 0.0
        in_maps.append({"xt": xt, "mask": mask})
    return in_maps


def assemble_output(results):
    outs = []
    for k in range(N_CORES):
        o = results[k]["out"]  # (OPOS, 256)
        outs.append(o.reshape(B_PER, T, C).transpose(0, 2, 1))
    return np.ascontiguousarray(np.concatenate(outs, axis=0))


def kernel(x):
    x = np.ascontiguousarray(x, dtype=np.float32)
    nc = _get_nc()
    in_maps = prepare_core_inputs(x)
    res = run_bass_kernel_spmd(nc, in_maps, core_ids=list(range(N_CORES)))
    return assemble_output(res.results)
